# revision 1
# baseline (speedup 1.0000x reference)
"""Trainium2 Bass kernel: ColumnParallelLinear + multi-adapter LoRA routing.

Computes out = x @ W^T + bias + B[aid[s]] @ (A[aid[s]] @ x[s]) for each token.

Distribution across 8 NeuronCores (one TRN2 chip):
  - base GEMM is tensor-parallel over d_out (sharding_hint): weight + bias
    sharded, each core emits out_base^T [512, S]
  - the LoRA delta is token-parallel: core i computes the delta for ITS
    1024-token slab across ALL d_out (A and B are tiny and replicated), so
    the rank-16 A-projection is computed once per token chip-wide instead
    of 8x replicated; no collectives needed — the host adds the two partial
    results while unsharding (out[s,o] = base[core o/512] + delta[core s/1024])
  - each core's token axis is ROTATED on the host so its own slab occupies
    the first two 512-token tiles; the xa matmuls then reuse the base
    x-strips already in SBUF (no extra x traffic, no prefetch stall), and
    the host un-rotates the base output during unsharding

Per-core kernel (all matmuls bf16, K=128 tiles, N=512 moving):
  - host pre-transposes x so the contraction dim lands on SBUF partitions
  - per-token adapter routing = precomputed {0,1} mask multiplied into the
    xa PSUM tile on the VectorE before the B_cat matmuls
  - bias is added during base PSUM->SBUF eviction (per-partition scalar add)
  - the 64 B_cat delta matmuls are drip-fed 2-per-base-m-tile so their
    PSUM-evict chain (ScalarE copy) never gates the PE
  - DMA emission is interleaved (w chunk k / x chunk k) with small leading
    chunks so the first matmul issues after ~256KB of DMA
"""

import os
import sys

import numpy as np

try:
    import ml_dtypes
except ImportError:  # pragma: no cover
    sys.path.insert(0, "/opt/trn_rl_repo")
    import ml_dtypes

_P = 128  # SBUF partitions / matmul tile edge
_NT = 512  # token tile (matmul moving free dim, one PSUM bank of fp32)
_LR = 128  # L * R = 8 * 16 adapter-rank rows
_N_CORES = 8

_NC_CACHE = {}
LAST_RESULTS = None  # BassKernelResults of the most recent run (for test.py)


def _import_concourse():
    try:
        import concourse  # noqa: F401
    except ImportError:  # pragma: no cover
        for p in ("/opt/trn_rl_repo", "/root/.axon_site/_ro/trn_rl_repo"):
            if os.path.isdir(p) and p not in sys.path:
                sys.path.insert(0, p)


def build_nc(d_in: int, d_loc: int, s_tokens: int, s_own: int, d_out: int):
    """Build + finalize the per-core Bass kernel.

    d_loc: output features of this core's base shard
    s_own: tokens in this core's LoRA-delta slab (the FIRST s_own tokens of
           the core's rotated token order)
    d_out: full output width (the delta covers all of it)
    """
    _import_concourse()
    import concourse.tile as tile
    from concourse import bacc, mybir

    P, NT, LR = _P, _NT, _LR
    n_kt = d_in // P
    n_mt = d_loc // P
    n_nt = s_tokens // NT
    n_ot = s_own // NT  # own-slab token tiles
    n_dt = d_out // P  # delta feature tiles
    assert all(v % P == 0 for v in (d_in, d_loc, d_out)) and s_tokens % NT == 0
    assert s_own % NT == 0 and n_ot <= n_nt

    nc = bacc.Bacc("TRN2", target_bir_lowering=False, debug=False)

    bf16 = mybir.dt.bfloat16
    f32 = mybir.dt.float32

    xT = nc.dram_tensor("xT", [d_in, s_tokens], bf16, kind="ExternalInput").ap()
    w_t = nc.dram_tensor("w_t", [d_in, d_loc], bf16, kind="ExternalInput").ap()
    a_t = nc.dram_tensor("a_t", [d_in, LR], bf16, kind="ExternalInput").ap()
    b_cat_t = nc.dram_tensor("b_cat_t", [LR, d_out], bf16, kind="ExternalInput").ap()
    mask_own = nc.dram_tensor("mask_own", [LR, s_own], bf16, kind="ExternalInput").ap()
    bias_pre = nc.dram_tensor("bias_pre", [P, n_mt], f32, kind="ExternalInput").ap()
    out_t = nc.dram_tensor("out_t", [d_loc, s_tokens], f32, kind="ExternalOutput").ap()
    delta_t = nc.dram_tensor("delta_t", [d_out, s_own], bf16, kind="ExternalOutput").ap()

    # [d_in, n] with d_in = kt*128 + p  ->  [p, kt, n]
    xT_v = xT.rearrange("(kt p) s -> p kt s", p=P)
    w_v = w_t.rearrange("(kt p) m -> p kt m", p=P)
    a_v = a_t.rearrange("(kt p) m -> p kt m", p=P)

    XCHUNK = 4  # k-tiles per x/w DMA chunk
    # finer chunks at the very start so the first matmul issues after ~256KB
    START_BOUNDS = [0, 1, 2, 3, 4]
    c = START_BOUNDS[-1]
    while c < n_kt:
        c = min(c + XCHUNK, n_kt)
        START_BOUNDS.append(c)
    START_BOUNDS = sorted(set(b for b in START_BOUNDS if b <= n_kt))

    with tile.TileContext(nc) as tc:
        with (
            tc.tile_pool(name="const", bufs=1) as const_pool,
            tc.tile_pool(name="xp", bufs=1) as x_pool,
            tc.tile_pool(name="outp", bufs=1) as out_pool,
            tc.tile_pool(name="psum", bufs=1, space="PSUM") as psum_pool,
        ):
            w_all = const_pool.tile([P, n_kt, d_loc], bf16)
            b_cat = const_pool.tile([P, n_dt, P], bf16)
            bias_sb = const_pool.tile([P, n_mt], f32)
            a_all = const_pool.tile([P, n_kt, LR], bf16)
            xa_sb = const_pool.tile([P, s_own], bf16)
            mask_sb = const_pool.tile([P, s_own], bf16)

            # Deferred LoRA-delta jobs, drip-fed between base m-tiles so the
            # PSUM-evict chain (ACT copy) never gates the PE.
            delta_jobs = []

            def emit_delta(k):
                for _ in range(k):
                    if not delta_jobs:
                        return
                    n, m = delta_jobs.pop(0)
                    dl_ps = psum_pool.tile(
                        [P, NT], f32, tag="dl", bufs=2, name=f"dl_ps{n}_{m}"
                    )
                    nc.tensor.matmul(
                        dl_ps[:],
                        b_cat[:, m, :],
                        xa_sb[:, n * NT : (n + 1) * NT],
                        start=True,
                        stop=True,
                    )
                    d_sb = out_pool.tile(
                        [P, NT], bf16, tag="d_sb", bufs=4, name=f"d_sb{n}_{m}"
                    )
                    nc.scalar.copy(d_sb[:], dl_ps[:])
                    nc.sync.dma_start(
                        delta_t[m * P : (m + 1) * P, n * NT : (n + 1) * NT], d_sb[:]
                    )

            def load_x_strip(j):
                x_strip = x_pool.tile(
                    [P, n_kt, NT], bf16, tag="x_strip", bufs=3, name=f"x_strip{j}"
                )
                tok = slice(j * NT, (j + 1) * NT)
                for c in range(0, n_kt, XCHUNK):
                    e = min(c + XCHUNK, n_kt)
                    nc.sync.dma_start(x_strip[:, c:e, :], xT_v[:, c:e, tok])
                return x_strip

            def evict_base(j, m, ps):
                tok0 = j * NT
                o_sb = out_pool.tile(
                    [P, NT], f32, tag="o_sb", bufs=6, name=f"o_sb{j}_{m}"
                )
                nc.vector.tensor_scalar_add(
                    out=o_sb[:], in0=ps[:], scalar1=bias_sb[:, m : m + 1]
                )
                # the very last evict+store trails the final matmul: spread it
                # over several DMA engines so the kernel tail shrinks
                pieces = {n_mt - 2: 2, n_mt - 1: 4}.get(m, 1) if j == n_nt - 1 else 1
                step = NT // pieces
                for q in range(pieces):
                    nc.sync.dma_start(
                        out_t[
                            m * P : (m + 1) * P,
                            tok0 + q * step : tok0 + (q + 1) * step,
                        ],
                        o_sb[:, q * step : (q + 1) * step],
                    )
                emit_delta(2)

            def base_ntile(j, x_strip, k_outer=False):
                if not k_outer:
                    for m in range(n_mt):
                        ps = psum_pool.tile(
                            [P, NT], f32, tag="base", bufs=4, name=f"ps{j}_{m}"
                        )
                        for kt in range(n_kt):
                            nc.tensor.matmul(
                                ps[:],
                                w_all[:, kt, m * P : (m + 1) * P],
                                x_strip[:, kt, :],
                                start=(kt == 0),
                                stop=(kt == n_kt - 1),
                            )
                        evict_base(j, m, ps)
                    return
                # k-outer: consume each k-chunk with one MM per m-tile the
                # moment it lands, so the DMA-paced prefix keeps the PE fed;
                # all n_mt PSUM banks accumulate concurrently
                pss = [
                    psum_pool.tile([P, NT], f32, tag="base", bufs=4, name=f"ps{j}_{m}")
                    for m in range(n_mt)
                ]
                for c, e in zip(START_BOUNDS, START_BOUNDS[1:]):
                    for m in range(n_mt):
                        for kt in range(c, e):
                            nc.tensor.matmul(
                                pss[m][:],
                                w_all[:, kt, m * P : (m + 1) * P],
                                x_strip[:, kt, :],
                                start=(kt == 0),
                                stop=(kt == n_kt - 1),
                            )
                for m in range(n_mt):
                    evict_base(j, m, pss[m])

            def xa_block(n, x_strip):
                # xa = A_all @ x^T for own-slab tile n, masked per-token;
                # queues that tile's 32 B_cat delta matmuls
                xa_ps = psum_pool.tile([P, NT], f32, tag="xa", bufs=2, name=f"xa_ps{n}")
                for kt in range(n_kt):
                    nc.tensor.matmul(
                        xa_ps[:],
                        a_all[:, kt, :],
                        x_strip[:, kt, :],
                        start=(kt == 0),
                        stop=(kt == n_kt - 1),
                    )
                nc.vector.tensor_mul(
                    out=xa_sb[:, n * NT : (n + 1) * NT],
                    in0=xa_ps[:],
                    in1=mask_sb[:, n * NT : (n + 1) * NT],
                )
                delta_jobs.extend((n, m) for m in range(n_dt))

            # ---- startup: interleave w chunks with x-strip j=0 chunks so the
            # first base matmuls have their operands after ~128KB of DMA;
            # the leading single-k-tile chunks are split in half across two
            # DMA engines to halve their arrival latency
            x_strip0 = x_pool.tile(
                [P, n_kt, NT], bf16, tag="x_strip", bufs=3, name="x_strip_first"
            )
            for c, e in zip(START_BOUNDS, START_BOUNDS[1:]):
                if e - c == 1:
                    q = d_loc // 4 if c == 0 else d_loc // 2
                    for h in range(0, d_loc, q):
                        nc.sync.dma_start(w_all[:, c, h : h + q], w_v[:, c, h : h + q])
                    q = NT // 4 if c == 0 else NT // 2
                    for h in range(0, NT, q):
                        nc.sync.dma_start(
                            x_strip0[:, c, h : h + q], xT_v[:, c, h : h + q]
                        )
                else:
                    nc.sync.dma_start(w_all[:, c:e, :], w_v[:, c:e, :])
                    nc.sync.dma_start(x_strip0[:, c:e, :], xT_v[:, c:e, 0:NT])
            nc.sync.dma_start(bias_sb[:], bias_pre)
            # warm the strip prefetch pipeline before any compute is emitted
            # (fresh pool slots -> these issue immediately on the Sync engine)
            strips = {0: x_strip0}
            K_OUTER = set()
            for j in (1, 2):
                if j < n_nt:
                    strips[j] = load_x_strip(j)
            # LoRA constants (a few MB; needed from ~40us in)
            for c in range(0, n_kt, XCHUNK):
                e = min(c + XCHUNK, n_kt)
                nc.sync.dma_start(a_all[:, c:e, :], a_v[:, c:e, :])
            nc.sync.dma_start(mask_sb[:], mask_own)
            for c in range(n_dt):
                nc.sync.dma_start(b_cat[:, c, :], b_cat_t[:, c * P : (c + 1) * P])

            for j in range(n_nt):
                x_strip = strips.pop(j) if j in strips else load_x_strip(j)
                base_ntile(j, x_strip, k_outer=j in K_OUTER)
                if j < n_ot:
                    xa_block(j, x_strip)
            while delta_jobs:
                emit_delta(len(delta_jobs))

    nc.finalize()
    return nc


def _get_nc(key):
    if key not in _NC_CACHE:
        _NC_CACHE[key] = build_nc(*key)
    return _NC_CACHE[key]


def make_in_maps(x, adapter_ids, weight, bias, A_buffer, B_buffer, n_cores=_N_CORES):
    """Host-side shard + layout prep. Returns (in_maps, shapes)."""
    bf16 = ml_dtypes.bfloat16
    x = np.asarray(x, dtype=np.float32)
    adapter_ids = np.asarray(adapter_ids, dtype=np.int32)
    weight = np.asarray(weight, dtype=np.float32)
    bias = np.asarray(bias, dtype=np.float32)
    A_buffer = np.asarray(A_buffer, dtype=np.float32)
    B_buffer = np.asarray(B_buffer, dtype=np.float32)

    S, D_IN = x.shape
    D_OUT = weight.shape[0]
    L, R, _ = A_buffer.shape
    d_loc = D_OUT // n_cores
    s_own = S // n_cores
    LR = L * R
    assert LR == _LR

    xT = np.ascontiguousarray(x.astype(bf16).T)  # [D_IN, S]
    a_t = np.ascontiguousarray(A_buffer.reshape(LR, D_IN).astype(bf16).T)
    b_cat_t = np.ascontiguousarray(
        B_buffer.transpose(0, 2, 1).reshape(LR, D_OUT).astype(bf16)
    )
    maskT = (np.arange(LR)[:, None] // R == adapter_ids[None, :]).astype(bf16)

    in_maps = []
    for i in range(n_cores):
        osl = slice(i * d_loc, (i + 1) * d_loc)
        w_t = np.ascontiguousarray(weight[osl].astype(bf16).T)  # [D_IN, d_loc]
        bias_pre = np.ascontiguousarray(bias[osl].reshape(d_loc // _P, _P).T)
        # rotate the token axis so core i's own slab comes first
        xT_rot = np.roll(xT, -i * s_own, axis=1) if i else xT
        in_maps.append(
            {
                "xT": np.ascontiguousarray(xT_rot),
                "w_t": w_t,
                "a_t": a_t,
                "b_cat_t": b_cat_t,
                "mask_own": np.ascontiguousarray(
                    maskT[:, i * s_own : (i + 1) * s_own]
                ),
                "bias_pre": bias_pre,
            }
        )
    return in_maps, (S, D_IN, D_OUT, d_loc, s_own)


def kernel(x, adapter_ids, weight, bias, A_buffer, B_buffer):
    global LAST_RESULTS
    _import_concourse()
    from concourse.bass_utils import run_bass_kernel_spmd

    in_maps, (S, D_IN, D_OUT, d_loc, s_own) = make_in_maps(
        x, adapter_ids, weight, bias, A_buffer, B_buffer
    )
    nc = _get_nc((D_IN, d_loc, S, s_own, D_OUT))
    LAST_RESULTS = run_bass_kernel_spmd(nc, in_maps, core_ids=list(range(_N_CORES)))
    res = LAST_RESULTS.results
    out = np.empty((S, D_OUT), dtype=np.float32)
    for i in range(_N_CORES):
        # un-rotate this core's token axis while scattering its base shard
        base = res[i]["out_t"]
        if i:
            base = np.roll(base, i * s_own, axis=1)
        out[:, i * d_loc : (i + 1) * d_loc] = base.T
    for i in range(_N_CORES):
        out[i * s_own : (i + 1) * s_own, :] += res[i]["delta_t"].T.astype(np.float32)
    return out



# revision 3
# speedup vs baseline: 1.2278x; 1.2278x over previous
"""Trainium2 Bass kernel: ColumnParallelLinear + multi-adapter LoRA routing.

Computes out = x @ W^T + bias + B[aid[s]] @ (A[aid[s]] @ x[s]) for each token.

Distribution across 8 NeuronCores (one TRN2 chip):
  - base GEMM is tensor-parallel over d_out (sharding_hint): weight + bias
    sharded, each core emits out_base^T [512, S]
  - the LoRA delta is token-parallel: core i computes the delta for ITS
    1024-token slab across ALL d_out (A and B are tiny and replicated); the
    host adds the two partial results while unsharding
  - each core's token axis is ROTATED on the host so its own slab occupies
    the first two 512-token tiles; the xa matmuls then reuse the base
    x-strips already in SBUF; the host un-rotates while unsharding

Precision strategy (rel-err budget 2e-2, measured 1.95e-2 end to end):
  - the first N8=12 of 32 k-tiles run in fp8 e4m3 with perf_mode=DoubleRow:
    one pair-instruction contracts K=256 in the time of a single bf16 MM
    (measured 216ns back-to-back, the full 2x) -> 6 pair MMs replace 12
  - fp8 operands are pre-scaled on the host (x*16, w*1024); the remaining
    bf16 k-tiles use weights pre-scaled by C=16384 so both parts accumulate
    in the SAME PSUM group; outputs are stored C-scaled in bf16 and the
    host divides while unsharding (no extra on-chip work)
  - the LoRA-A projection uses the same k-split; B-matmuls stay bf16

Schedule:
  - startup: fp8 w + fp8 x-strip0 land first (1.5MB in a few large DMAs at
    ~400GB/s aggregate - single dma_starts stripe over all 16 DMA engines);
    j=0 runs "k-outer" so every arriving k-chunk immediately feeds all 4
    m-tile PSUM groups -> PE is busy ~2us after the DMA preamble
  - bias is added during base PSUM->SBUF eviction (per-partition scalar add)
  - the 64 B_cat delta matmuls are drip-fed 2-per-base-evict
  - outputs are stored as scaled bf16 (halves store traffic; the final
    stores are split into pieces so the kernel tail stays short)
"""

import os
import sys

import numpy as np

try:
    import ml_dtypes
except ImportError:  # pragma: no cover
    sys.path.insert(0, "/opt/trn_rl_repo")
    import ml_dtypes

_P = 128  # SBUF partitions / matmul tile edge
_NT = 512  # token tile (matmul moving free dim, one PSUM bank of fp32)
_LR = 128  # L * R = 8 * 16 adapter-rank rows
_N_CORES = 8
_N8 = 12  # k-tiles (of 32) computed in fp8 e4m3 DoubleRow
_SX = np.float32(16.0)  # x fp8 scale
_SW = np.float32(1024.0)  # w fp8 scale
_SA = np.float32(1024.0)  # A fp8 scale

_NC_CACHE = {}
LAST_RESULTS = None  # BassKernelResults of the most recent run (for test.py)


def _import_concourse():
    try:
        import concourse  # noqa: F401
    except ImportError:  # pragma: no cover
        for p in ("/opt/trn_rl_repo", "/root/.axon_site/_ro/trn_rl_repo"):
            if os.path.isdir(p) and p not in sys.path:
                sys.path.insert(0, p)


def build_nc(d_in: int, d_loc: int, s_tokens: int, s_own: int, d_out: int):
    """Build + finalize the per-core Bass kernel.

    d_loc: output features of this core's base shard
    s_own: tokens in this core's LoRA-delta slab (the FIRST s_own tokens of
           the core's rotated token order)
    d_out: full output width (the delta covers all of it)
    """
    _import_concourse()
    import concourse.tile as tile
    from concourse import bacc, mybir

    P, NT, LR, N8 = _P, _NT, _LR, _N8
    n_kt = d_in // P
    NB = n_kt - N8  # bf16 k-tiles
    NPR = N8 // 2  # fp8 DoubleRow pairs
    n_mt = d_loc // P
    n_nt = s_tokens // NT
    n_ot = s_own // NT  # own-slab token tiles
    n_dt = d_out // P  # delta feature tiles
    assert all(v % P == 0 for v in (d_in, d_loc, d_out)) and s_tokens % NT == 0
    assert s_own % NT == 0 and n_ot <= n_nt and N8 % 2 == 0

    nc = bacc.Bacc("TRN2", target_bir_lowering=False, debug=False)

    bf16 = mybir.dt.bfloat16
    f8 = mybir.dt.float8e4
    f32 = mybir.dt.float32
    DR = mybir.MatmulPerfMode.DoubleRow

    x8T = nc.dram_tensor("x8T", [N8 * P, s_tokens], f8, kind="ExternalInput").ap()
    xbT = nc.dram_tensor("xbT", [NB * P, s_tokens], bf16, kind="ExternalInput").ap()
    w8_t = nc.dram_tensor("w8_t", [N8 * P, d_loc], f8, kind="ExternalInput").ap()
    wb_t = nc.dram_tensor("wb_t", [NB * P, d_loc], bf16, kind="ExternalInput").ap()
    a8_t = nc.dram_tensor("a8_t", [N8 * P, LR], f8, kind="ExternalInput").ap()
    ab_t = nc.dram_tensor("ab_t", [NB * P, LR], bf16, kind="ExternalInput").ap()
    b_cat_t = nc.dram_tensor("b_cat_t", [LR, d_out], bf16, kind="ExternalInput").ap()
    mask_own = nc.dram_tensor("mask_own", [LR, s_own], bf16, kind="ExternalInput").ap()
    bias_pre = nc.dram_tensor("bias_pre", [P, n_mt], f32, kind="ExternalInput").ap()
    out_t = nc.dram_tensor("out_t", [d_loc, s_tokens], bf16, kind="ExternalOutput").ap()
    delta_t = nc.dram_tensor("delta_t", [d_out, s_own], bf16, kind="ExternalOutput").ap()

    # [kt*128 + p, n] -> [p, kt, n]
    x8_v = x8T.rearrange("(kt p) s -> p kt s", p=P)
    xb_v = xbT.rearrange("(kt p) s -> p kt s", p=P)
    w8_v = w8_t.rearrange("(kt p) m -> p kt m", p=P)
    wb_v = wb_t.rearrange("(kt p) m -> p kt m", p=P)
    a8_v = a8_t.rearrange("(kt p) m -> p kt m", p=P)
    ab_v = ab_t.rearrange("(kt p) m -> p kt m", p=P)

    # startup chunking: fp8 pairs first (fine-grained so the PE starts ~2us
    # in), then bf16 in 4-k-tile chunks
    PAIR_CHUNKS = [(0, 1), (1, 2)] + [(c, min(c + 2, NPR)) for c in range(2, NPR, 2)]
    BF_CHUNKS = [(c, min(c + 4, NB)) for c in range(0, NB, 4)]

    with tile.TileContext(nc) as tc:
        with (
            tc.tile_pool(name="const", bufs=1) as const_pool,
            tc.tile_pool(name="xp", bufs=1) as x_pool,
            tc.tile_pool(name="outp", bufs=1) as out_pool,
            tc.tile_pool(name="psum", bufs=1, space="PSUM") as psum_pool,
        ):
            w8_all = const_pool.tile([P, N8, d_loc], f8)
            wb_all = const_pool.tile([P, NB, d_loc], bf16)
            b_cat = const_pool.tile([P, d_out], bf16)
            bias_sb = const_pool.tile([P, n_mt], f32)
            a8_all = const_pool.tile([P, N8, LR], f8)
            ab_all = const_pool.tile([P, NB, LR], bf16)
            xa_sb = const_pool.tile([P, s_own], bf16)
            mask_sb = const_pool.tile([P, s_own], bf16)

            # Deferred LoRA-delta jobs, drip-fed between base m-tiles.
            delta_jobs = []

            def emit_delta(k):
                for _ in range(k):
                    if not delta_jobs:
                        return
                    n, m = delta_jobs.pop(0)
                    dl_ps = psum_pool.tile(
                        [P, NT], f32, tag="dl", bufs=2, name=f"dl_ps{n}_{m}"
                    )
                    nc.tensor.matmul(
                        dl_ps[:],
                        b_cat[:, m * P : (m + 1) * P],
                        xa_sb[:, n * NT : (n + 1) * NT],
                        start=True,
                        stop=True,
                    )
                    d_sb = out_pool.tile(
                        [P, NT], bf16, tag="d_sb", bufs=4, name=f"d_sb{n}_{m}"
                    )
                    nc.scalar.copy(d_sb[:], dl_ps[:])
                    nc.sync.dma_start(
                        delta_t[m * P : (m + 1) * P, n * NT : (n + 1) * NT], d_sb[:]
                    )

            def load_x_strip(j):
                tok = slice(j * NT, (j + 1) * NT)
                x8s = x_pool.tile([P, N8, NT], f8, tag="x8s", bufs=3, name=f"x8s{j}")
                xbs = x_pool.tile([P, NB, NT], bf16, tag="xbs", bufs=3, name=f"xbs{j}")
                nc.sync.dma_start(x8s[:], x8_v[:, :, tok])
                for c, e in ((0, NB // 2), (NB // 2, NB)):
                    nc.sync.dma_start(xbs[:, c:e, :], xb_v[:, c:e, tok])
                return x8s, xbs

            def evict_base(j, m, ps):
                tok0 = j * NT
                o_sb = out_pool.tile(
                    [P, NT], bf16, tag="o_sb", bufs=6, name=f"o_sb{j}_{m}"
                )
                nc.vector.tensor_scalar_add(
                    out=o_sb[:], in0=ps[:], scalar1=bias_sb[:, m : m + 1]
                )
                # the very last evict+store trails the final matmul: spread it
                # over several DMA engines so the kernel tail shrinks
                pieces = {n_mt - 2: 2, n_mt - 1: 4}.get(m, 1) if j == n_nt - 1 else 1
                step = NT // pieces
                for q in range(pieces):
                    nc.sync.dma_start(
                        out_t[
                            m * P : (m + 1) * P,
                            tok0 + q * step : tok0 + (q + 1) * step,
                        ],
                        o_sb[:, q * step : (q + 1) * step],
                    )
                emit_delta(2)

            def base_group_mms(ps, m, x8s, xbs, pr_range, kb_range):
                for pr in pr_range:
                    nc.tensor.matmul(
                        ps[:],
                        w8_all[:, 2 * pr : 2 * pr + 2, m * P : (m + 1) * P],
                        x8s[:, 2 * pr : 2 * pr + 2, :],
                        start=(pr == 0),
                        stop=False,
                        perf_mode=DR,
                    )
                for kt in kb_range:
                    nc.tensor.matmul(
                        ps[:],
                        wb_all[:, kt, m * P : (m + 1) * P],
                        xbs[:, kt, :],
                        start=False,
                        stop=(kt == NB - 1),
                    )

            def base_ntile(j, x8s, xbs, k_outer=False):
                if not k_outer:
                    for m in range(n_mt):
                        ps = psum_pool.tile(
                            [P, NT], f32, tag="base", bufs=4, name=f"ps{j}_{m}"
                        )
                        base_group_mms(ps, m, x8s, xbs, range(NPR), range(NB))
                        evict_base(j, m, ps)
                    return
                # k-outer: consume each k-chunk across all m-tiles the moment
                # it lands, so the DMA-paced startup keeps the PE fed
                pss = [
                    psum_pool.tile([P, NT], f32, tag="base", bufs=4, name=f"ps{j}_{m}")
                    for m in range(n_mt)
                ]
                for c, e in PAIR_CHUNKS:
                    for m in range(n_mt):
                        base_group_mms(pss[m], m, x8s, xbs, range(c, e), ())
                for c, e in BF_CHUNKS:
                    for m in range(n_mt):
                        base_group_mms(pss[m], m, x8s, xbs, (), range(c, e))
                for m in range(n_mt):
                    evict_base(j, m, pss[m])

            def xa_block(n, x8s, xbs):
                # xa = A_all @ x^T for own-slab tile n, masked per-token;
                # queues that tile's 32 B_cat delta matmuls
                xa_ps = psum_pool.tile([P, NT], f32, tag="xa", bufs=2, name=f"xa_ps{n}")
                for pr in range(NPR):
                    nc.tensor.matmul(
                        xa_ps[:],
                        a8_all[:, 2 * pr : 2 * pr + 2, :],
                        x8s[:, 2 * pr : 2 * pr + 2, :],
                        start=(pr == 0),
                        stop=False,
                        perf_mode=DR,
                    )
                for kt in range(NB):
                    nc.tensor.matmul(
                        xa_ps[:],
                        ab_all[:, kt, :],
                        xbs[:, kt, :],
                        start=False,
                        stop=(kt == NB - 1),
                    )
                nc.vector.tensor_mul(
                    out=xa_sb[:, n * NT : (n + 1) * NT],
                    in0=xa_ps[:],
                    in1=mask_sb[:, n * NT : (n + 1) * NT],
                )
                delta_jobs.extend((n, m) for m in range(n_dt))

            # ---- startup DMA: fp8 w + fp8 strip0 first (pair-granular so the
            # first DR matmuls issue ~2us in), then bf16 w / strip0 chunks,
            # then prefetch strips j=1,2 and the LoRA constants
            x8s0 = x_pool.tile([P, N8, NT], f8, tag="x8s", bufs=3, name="x8s_first")
            xbs0 = x_pool.tile([P, NB, NT], bf16, tag="xbs", bufs=3, name="xbs_first")
            for c, e in PAIR_CHUNKS:
                nc.sync.dma_start(
                    w8_all[:, 2 * c : 2 * e, :], w8_v[:, 2 * c : 2 * e, :]
                )
                nc.sync.dma_start(
                    x8s0[:, 2 * c : 2 * e, :], x8_v[:, 2 * c : 2 * e, 0:NT]
                )
            nc.sync.dma_start(bias_sb[:], bias_pre)
            for c, e in BF_CHUNKS:
                nc.sync.dma_start(wb_all[:, c:e, :], wb_v[:, c:e, :])
                nc.sync.dma_start(xbs0[:, c:e, :], xb_v[:, c:e, 0:NT])
            strips = {0: (x8s0, xbs0)}
            for j in (1, 2):
                if j < n_nt:
                    strips[j] = load_x_strip(j)
            # LoRA constants (needed from ~30us in)
            nc.sync.dma_start(a8_all[:], a8_v)
            nc.sync.dma_start(ab_all[:], ab_v)
            nc.sync.dma_start(mask_sb[:], mask_own)
            nc.sync.dma_start(b_cat[:], b_cat_t)

            for j in range(n_nt):
                x8s, xbs = strips.pop(j) if j in strips else load_x_strip(j)
                base_ntile(j, x8s, xbs, k_outer=(j == 0))
                if j < n_ot:
                    xa_block(j, x8s, xbs)
            while delta_jobs:
                emit_delta(len(delta_jobs))

    nc.finalize()
    return nc


def _get_nc(key):
    if key not in _NC_CACHE:
        _NC_CACHE[key] = build_nc(*key)
    return _NC_CACHE[key]


def make_in_maps(x, adapter_ids, weight, bias, A_buffer, B_buffer, n_cores=_N_CORES):
    """Host-side shard + layout + quantization prep. Returns (in_maps, shapes)."""
    bf16 = ml_dtypes.bfloat16
    e4 = ml_dtypes.float8_e4m3
    x = np.asarray(x, dtype=np.float32)
    adapter_ids = np.asarray(adapter_ids, dtype=np.int32)
    weight = np.asarray(weight, dtype=np.float32)
    bias = np.asarray(bias, dtype=np.float32)
    A_buffer = np.asarray(A_buffer, dtype=np.float32)
    B_buffer = np.asarray(B_buffer, dtype=np.float32)

    S, D_IN = x.shape
    D_OUT = weight.shape[0]
    L, R, _ = A_buffer.shape
    d_loc = D_OUT // n_cores
    s_own = S // n_cores
    LR = L * R
    K8 = _N8 * _P
    C = _SX * _SW
    C2 = _SX * _SA
    assert LR == _LR

    def q8(a, scale):
        return np.clip(a * scale, -240.0, 240.0).astype(e4)

    x8T = np.ascontiguousarray(q8(x[:, :K8], _SX).T)  # [K8, S] fp8
    xbT = np.ascontiguousarray(x[:, K8:].astype(bf16).T)  # [D_IN-K8, S] bf16
    Af = A_buffer.reshape(LR, D_IN)
    a8_t = np.ascontiguousarray(q8(Af[:, :K8], _SA).T)
    ab_t = np.ascontiguousarray((Af[:, K8:] * C2).astype(bf16).T)
    b_cat_t = np.ascontiguousarray(
        B_buffer.transpose(0, 2, 1).reshape(LR, D_OUT).astype(bf16)
    )
    maskT = (np.arange(LR)[:, None] // R == adapter_ids[None, :]).astype(bf16)

    in_maps = []
    for i in range(n_cores):
        osl = slice(i * d_loc, (i + 1) * d_loc)
        w8_t = np.ascontiguousarray(q8(weight[osl, :K8], _SW).T)  # [K8, d_loc]
        wb_t = np.ascontiguousarray((weight[osl, K8:] * C).astype(bf16).T)
        bias_pre = np.ascontiguousarray((bias[osl] * C).reshape(d_loc // _P, _P).T)
        # rotate the token axis so core i's own slab comes first
        sh = -i * s_own
        in_maps.append(
            {
                "x8T": np.ascontiguousarray(np.roll(x8T, sh, axis=1)) if i else x8T,
                "xbT": np.ascontiguousarray(np.roll(xbT, sh, axis=1)) if i else xbT,
                "w8_t": w8_t,
                "wb_t": wb_t,
                "a8_t": a8_t,
                "ab_t": ab_t,
                "b_cat_t": b_cat_t,
                "mask_own": np.ascontiguousarray(
                    maskT[:, i * s_own : (i + 1) * s_own]
                ),
                "bias_pre": bias_pre,
            }
        )
    return in_maps, (S, D_IN, D_OUT, d_loc, s_own)


def kernel(x, adapter_ids, weight, bias, A_buffer, B_buffer):
    global LAST_RESULTS
    _import_concourse()
    from concourse.bass_utils import run_bass_kernel_spmd

    in_maps, (S, D_IN, D_OUT, d_loc, s_own) = make_in_maps(
        x, adapter_ids, weight, bias, A_buffer, B_buffer
    )
    nc = _get_nc((D_IN, d_loc, S, s_own, D_OUT))
    LAST_RESULTS = run_bass_kernel_spmd(nc, in_maps, core_ids=list(range(_N_CORES)))
    res = LAST_RESULTS.results
    inv_c = np.float32(1.0) / (_SX * _SW)
    inv_c2 = np.float32(1.0) / (_SX * _SA)
    out = np.empty((S, D_OUT), dtype=np.float32)
    for i in range(_N_CORES):
        # un-rotate this core's token axis while scattering its base shard
        base = res[i]["out_t"].astype(np.float32) * inv_c
        if i:
            base = np.roll(base, i * s_own, axis=1)
        out[:, i * d_loc : (i + 1) * d_loc] = base.T
    for i in range(_N_CORES):
        out[i * s_own : (i + 1) * s_own, :] += (
            res[i]["delta_t"].T.astype(np.float32) * inv_c2
        )
    return out
